# revision 1
# baseline (speedup 1.0000x reference)
"""Trainium2 Bass kernel for the ComplexRenderer problem.

field[n] = sum_p a_p * exp(-0.5*(x_n-mu_p)^T diag(1/s_p^2) (x_n-mu_p))
                 * exp(i*(phi_p + k*|x_n-mu_p|))

Per-core (data-parallel over query points, 8 cores):
  - maha/d2 quadratic forms as K=7 GEMMs over features [x^2(3), x(3), 1],
    primitives on partitions (16 tiles of 128), points on free dim.
    K=7 wastes 121/128 PE rows, so four primitive-tiles' GEMMs are packed
    into disjoint 32-row groups of the systolic array (tile_position) and
    run concurrently (~3x measured on such shapes).
  - amplitude folded into maha constant row via -2*ln(a_p):
    w = exp(-0.5*maha') = a_p * exp(-0.5*maha)
  - phase in 1/65536-turn units: Bd pre-scaled so sqrt(d2_gemm) = theta
    in units; ScalarE Sqrt writes int32 (round-on-cast). Clamp of the
    GEMM's tiny negative cancellation noise is a Relu pass on ScalarE.
    Exact integer range reduction on DVE:
        f = (theta_i + phi') & 65535 ;  Sin(f*2pi/65536 - pi) = -sin/-cos
  - products w*(-cos), w*(-sin) in fp16 on DVE; reduction over primitives
    by TensorE matmul with a [-1] column, accumulated in PSUM over the 16
    primitive tiles.
  - ScalarE work batched by activation-table set (exp / sqrt / sin) with
    explicit scheduling edges between phases so the ~2.7us ACT_TABLE_LOADs
    don't thrash.
"""

import numpy as np

N_POINTS = 32768
N_PRIMS = 2048
N_CORES = 8
C_LIGHT = 299792458.0
P_TILES = N_PRIMS // 128  # 16
QUADS = P_TILES // 4      # 4 row-group-packed GEMM quads


def prep_inputs(query_points, positions, scales, amplitudes, phases, frequency):
    q = np.asarray(query_points, np.float64)
    pos = np.asarray(positions, np.float64)
    sc = np.asarray(scales, np.float64)
    amp = np.asarray(amplitudes, np.float64)
    ph = np.asarray(phases, np.float64)

    k32 = np.float32(2.0 * np.pi) * np.float32(frequency) / np.float32(C_LIGHT)
    k = float(k32)

    n = q.shape[0]
    at = np.empty((7, n), np.float64)
    at[0:3] = (q * q).T
    at[3:6] = q.T
    at[6] = 1.0

    inv_var = 1.0 / (sc * sc)
    bm = np.empty((7, N_PRIMS), np.float64)
    bm[0:3] = inv_var.T
    bm[3:6] = (-2.0 * pos * inv_var).T
    bm[6] = np.sum(pos * pos * inv_var, axis=1) - 2.0 * np.log(
        np.maximum(amp, 1e-35)
    )

    s = 65536.0 * k / (2.0 * np.pi)  # phase units per metre
    sqs = s * s
    bd = np.empty((7, N_PRIMS), np.float64)
    bd[0:3] = sqs
    bd[3:6] = (-2.0 * sqs) * pos.T
    bd[6] = sqs * np.sum(pos * pos, axis=1)

    # pack [7, 2048] coefficient mats into 32-row groups: quad q, member i
    # (p_tile t = 4q+i) lives at partitions 32i..32i+6, cols q*128..
    f32 = np.float32
    bmq = np.zeros((128, QUADS * 128), f32)
    bdq = np.zeros((128, QUADS * 128), f32)
    for t in range(P_TILES):
        qd, i = divmod(t, 4)
        rows = slice(32 * i, 32 * i + 7)
        cols = slice(qd * 128, (qd + 1) * 128)
        bmq[rows, cols] = bm[:, t * 128 : (t + 1) * 128]
        bdq[rows, cols] = bd[:, t * 128 : (t + 1) * 128]

    phi0 = np.round(65536.0 * (ph / (2.0 * np.pi) + 1.0))
    phi1 = phi0 + 16384.0
    phi0_c = np.ascontiguousarray(phi0.reshape(P_TILES, 128).T)
    phi1_c = np.ascontiguousarray(phi1.reshape(P_TILES, 128).T)

    return (
        np.ascontiguousarray(at, dtype=f32),
        bmq,
        bdq,
        phi0_c.astype(f32),
        phi1_c.astype(f32),
    )


def build_program(n_per_core, n_chunk=1024, sin_group=2):
    from contextlib import ExitStack

    import concourse.bacc as bacc
    import concourse.tile as tile
    import concourse.mybir as mybir
    from concourse.tile_rust import add_dep_helper

    dt = mybir.dt
    AF = mybir.ActivationFunctionType
    OP = mybir.AluOpType

    assert n_per_core % n_chunk == 0
    assert n_chunk % 512 == 0
    n_chunks = n_per_core // n_chunk
    n_subs = n_chunk // 512
    assert P_TILES % sin_group == 0
    gw = sin_group * n_chunk

    sin_scale = float(2.0 * np.pi / 65536.0)
    sin_bias = float(-np.pi)

    nc = bacc.Bacc("TRN2", target_bir_lowering=False, debug=False)

    at_d = nc.dram_tensor("at_in", [7, n_per_core], dt.float32, kind="ExternalInput")
    bm_d = nc.dram_tensor("bm_in", [128, QUADS * 128], dt.float32, kind="ExternalInput")
    bd_d = nc.dram_tensor("bd_in", [128, QUADS * 128], dt.float32, kind="ExternalInput")
    p0_d = nc.dram_tensor("phi0_in", [128, P_TILES], dt.float32, kind="ExternalInput")
    p1_d = nc.dram_tensor("phi1_in", [128, P_TILES], dt.float32, kind="ExternalInput")
    or_d = nc.dram_tensor("out_re", [1, n_per_core], dt.float32, kind="ExternalOutput")
    oi_d = nc.dram_tensor("out_im", [1, n_per_core], dt.float32, kind="ExternalOutput")

    with tile.TileContext(nc) as tc, ExitStack() as ctx:
        const = ctx.enter_context(tc.tile_pool(name="const", bufs=1))
        atpool = ctx.enter_context(tc.tile_pool(name="atp", bufs=2))
        wpool = ctx.enter_context(tc.tile_pool(name="wp", bufs=1))
        tipool = ctx.enter_context(tc.tile_pool(name="tip", bufs=1))
        d2pool = ctx.enter_context(tc.tile_pool(name="d2p", bufs=3))
        fgpool = ctx.enter_context(tc.tile_pool(name="fgp", bufs=3))
        cspool = ctx.enter_context(tc.tile_pool(name="csp", bufs=3))
        prpool = ctx.enter_context(tc.tile_pool(name="prp", bufs=4))
        opool = ctx.enter_context(tc.tile_pool(name="op", bufs=2))
        mmpool = ctx.enter_context(tc.tile_pool(name="mmp", bufs=6, space="PSUM"))
        accpool = ctx.enter_context(tc.tile_pool(name="accp", bufs=2, space="PSUM"))

        bm_sb = const.tile([128, QUADS * 128], dt.float32)
        nc.sync.dma_start(bm_sb[:], bm_d.ap())
        bd_sb = const.tile([128, QUADS * 128], dt.float32)
        nc.sync.dma_start(bd_sb[:], bd_d.ap())
        p0_sb = const.tile([128, P_TILES], dt.float32)
        nc.sync.dma_start(p0_sb[:], p0_d.ap())
        p1_sb = const.tile([128, P_TILES], dt.float32)
        nc.sync.dma_start(p1_sb[:], p1_d.ap())
        negones = const.tile([128, 1], dt.float16)
        nc.gpsimd.memset(negones[:], -1.0)
        sinbias = const.tile([128, 1], dt.float32)
        nc.gpsimd.memset(sinbias[:], sin_bias)

        prev_act = [None]  # last ACT instruction of the previous table phase

        def act(first_of_phase, *args, **kw):
            ins = nc.scalar.activation(*args, **kw)
            if first_of_phase and prev_act[0] is not None:
                add_dep_helper(
                    ins.ins, prev_act[0].ins, sync=False, reason="act set order"
                )
            return ins

        for c in range(n_chunks):
            ccols = slice(c * n_chunk, (c + 1) * n_chunk)
            at_c = atpool.tile([128, n_chunk], dt.float32, tag="at")
            for i in range(4):
                nc.sync.dma_start(at_c[32 * i : 32 * i + 7, :], at_d.ap()[:, ccols])
            w_sb = wpool.tile([128, P_TILES * n_chunk], dt.float16, tag="w")
            thi = tipool.tile([128, P_TILES * n_chunk], dt.int32, tag="ti")

            # ---- phase A: maha quad-GEMMs + exp (exp table set) ----
            last = None
            for s in range(n_subs):
                scols = slice(s * 512, (s + 1) * 512)
                for qd in range(QUADS):
                    mms = [
                        mmpool.tile([128, 512], dt.float32, tag="mm",
                                    name=f"mmA{s}{qd}{j}")
                        for j in range(4)
                    ]
                    for i in range(4):
                        nc.tensor.matmul(
                            mms[i][:],
                            bm_sb[32 * i : 32 * i + 7, qd * 128 : (qd + 1) * 128],
                            at_c[32 * i : 32 * i + 7, scols],
                            start=True,
                            stop=True,
                            tile_position=(32 * i, 0),
                        )
                    for i in range(4):
                        t = 4 * qd + i
                        wcols = slice(t * n_chunk + s * 512, t * n_chunk + (s + 1) * 512)
                        last = act(
                            (s, qd, i) == (0, 0, 0),
                            w_sb[:, wcols], mms[i][:], AF.Exp, scale=-0.5,
                        )
            prev_act[0] = last

            # ---- phase B: d2 quad-GEMMs + relu-clamp + sqrt->i32 ----
            last = None
            firstb = True
            for s in range(n_subs):
                scols = slice(s * 512, (s + 1) * 512)
                for qd in range(QUADS):
                    mms = [
                        mmpool.tile([128, 512], dt.float32, tag="mm",
                                    name=f"mmB{s}{qd}{j}")
                        for j in range(4)
                    ]
                    for i in range(4):
                        nc.tensor.matmul(
                            mms[i][:],
                            bd_sb[32 * i : 32 * i + 7, qd * 128 : (qd + 1) * 128],
                            at_c[32 * i : 32 * i + 7, scols],
                            start=True,
                            stop=True,
                            tile_position=(32 * i, 0),
                        )
                    for i in range(4):
                        t = 4 * qd + i
                        tcols = slice(t * n_chunk + s * 512, t * n_chunk + (s + 1) * 512)
                        last = act(firstb, thi[:, tcols], mms[i][:], AF.Sqrt)
                        firstb = False
            prev_act[0] = last

            # ---- phase C: wrap + sin + products + reduction (sin set) ----
            accs = [accpool.tile([64, 512], dt.float32, tag="acc", name=f"acc{h}")
                    for h in range(2)]
            firstc = True
            for g in range(P_TILES // sin_group):
                f0g = fgpool.tile([128, gw], dt.int32, tag="fg")
                f1g = fgpool.tile([128, gw], dt.int32, tag="fg")
                for u in range(sin_group):
                    t = g * sin_group + u
                    tcols = slice(t * n_chunk, (t + 1) * n_chunk)
                    ucols = slice(u * n_chunk, (u + 1) * n_chunk)
                    nc.vector.tensor_scalar(
                        f0g[:, ucols], thi[:, tcols], p0_sb[:, t : t + 1],
                        None, OP.add,
                    )
                    nc.vector.tensor_scalar(
                        f1g[:, ucols], thi[:, tcols], p1_sb[:, t : t + 1],
                        None, OP.add,
                    )
                nc.vector.tensor_scalar(f0g[:], f0g[:], 65535, None, OP.bitwise_and)
                nc.vector.tensor_scalar(f1g[:], f1g[:], 65535, None, OP.bitwise_and)
                sn = cspool.tile([128, gw], dt.float16, tag="cs")
                cs = cspool.tile([128, gw], dt.float16, tag="cs")
                act(firstc, sn[:], f0g[:], AF.Sin, scale=sin_scale, bias=sinbias[:])
                firstc = False
                last = act(False, cs[:], f1g[:], AF.Sin, scale=sin_scale,
                           bias=sinbias[:])
                gcols = slice(g * gw, (g + 1) * gw)
                wc = prpool.tile([128, gw], dt.float16, tag="pr")
                ws = prpool.tile([128, gw], dt.float16, tag="pr")
                nc.vector.tensor_mul(wc[:], w_sb[:, gcols], cs[:])
                nc.vector.tensor_mul(ws[:], w_sb[:, gcols], sn[:])
                for u in range(sin_group):
                    t = g * sin_group + u
                    first = t == 0
                    lastmm = t == P_TILES - 1
                    for h in range(n_subs):
                        ghs = slice(u * n_chunk + h * 512, u * n_chunk + (h + 1) * 512)
                        nc.tensor.matmul(
                            accs[h][0:1, :], negones[:], wc[:, ghs],
                            start=first, stop=lastmm, tile_position=(0, 0),
                        )
                        nc.tensor.matmul(
                            accs[h][32:33, :], negones[:], ws[:, ghs],
                            start=first, stop=lastmm, tile_position=(0, 32),
                        )
            o_re = opool.tile([1, n_chunk], dt.float32, tag="o")
            o_im = opool.tile([1, n_chunk], dt.float32, tag="o")
            for h in range(n_subs):
                hs = slice(h * 512, (h + 1) * 512)
                nc.vector.tensor_copy(o_re[:, hs], accs[h][0:1, :])
                nc.vector.tensor_copy(o_im[:, hs], accs[h][32:33, :])
            prev_act[0] = last
            nc.sync.dma_start(or_d.ap()[:, ccols], o_re[:])
            nc.sync.dma_start(oi_d.ap()[:, ccols], o_im[:])

    nc.compile()
    names = dict(
        at=at_d.name, bm=bm_d.name, bd=bd_d.name,
        p0=p0_d.name, p1=p1_d.name, out_re=or_d.name, out_im=oi_d.name,
    )
    return nc, names


_CACHE = {}
LAST_RESULTS = None


def kernel(query_points, positions, scales, amplitudes, phases, frequency):
    global LAST_RESULTS
    from concourse import bass_utils

    at, bmq, bdq, p0, p1 = prep_inputs(
        query_points, positions, scales, amplitudes, phases, frequency
    )
    n = at.shape[1]
    assert n % N_CORES == 0
    npc = n // N_CORES

    key = (npc,)
    if key not in _CACHE:
        _CACHE[key] = build_program(npc)
    nc, names = _CACHE[key]

    in_maps = []
    for i in range(N_CORES):
        in_maps.append(
            {
                names["at"]: np.ascontiguousarray(at[:, i * npc : (i + 1) * npc]),
                names["bm"]: bmq,
                names["bd"]: bdq,
                names["p0"]: p0,
                names["p1"]: p1,
            }
        )

    res = bass_utils.run_bass_kernel_spmd(nc, in_maps, core_ids=list(range(N_CORES)))
    LAST_RESULTS = res
    re = np.concatenate([r[names["out_re"]][0] for r in res.results])
    im = np.concatenate([r[names["out_im"]][0] for r in res.results])
    return (re + 1j * im).astype(np.complex64)

